# revision 30
# baseline (speedup 1.0000x reference)
"""Causal Performer (linear attention) Trainium2 kernel.

Full inputs in, full output out. Sharding: data-parallel over batch (B=2)
x tensor-parallel over heads (16 heads -> 4 per core), 8 cores total.
Each core computes a partial output projection (its heads' rows of w_o);
the host sums the 4 partials per batch element.

Math notes vs the reference:
  - BLOCK_H=BLOCK_W=1 makes the blockwise selection an inclusive causal
    prefix scan -> standard chunked linear attention.
  - qp normalization cancels in numerator/denominator (up to an EPS-scale
    term ~1e-5 relative), so qp is used unnormalized.
  - kp normalization b[j] = 1/(sum_f kp + EPS) is folded into the value
    matrix v1[j] = b[j] * [v[j] | 1], which feeds both the intra-chunk
    (A_masked @ v1) and the running-state (kp^T @ v1) paths exactly once.
  - The feature-map projection is fused on the host:
    s = (q @ w_q^T)_head @ omega^T = q @ W2 with W2 = w_q_head^T @ omega^T.
"""

import numpy as np

L, D = 4096, 1024
H_ALL, DK, F = 16, 64, 16
HC = 4              # heads per core
M = HC * DK         # 256 head-dims per core
EPS = 1e-6
C = 128             # scan chunk
C2 = 512            # projection / load chunk
NC2 = L // C2       # 8
SUB = C2 // C       # 4
NCH = L // C        # 32 scan chunks
N_CORES = 8

_CACHE = {}


def _tile_T(x16):
    # [L, D] f16 -> [NC2, 128, KD, C2] with [c2, p, kd, l] = x[c2*C2+l, kd*128+p]
    return np.ascontiguousarray(
        x16.reshape(NC2, C2, D // 128, 128).transpose(0, 3, 2, 1)
    )


def _build_bass():
    from contextlib import ExitStack

    import concourse.bacc as bacc
    import concourse.tile as tile
    from concourse import mybir
    from concourse.masks import make_identity

    f16 = mybir.dt.float16
    f32 = mybir.dt.float32

    nc = bacc.Bacc("TRN2", debug=False, num_devices=1)
    f8 = mybir.dt.float8e4
    KDc = D // 128
    # pre-tiled transposed inputs: [c2, p, kd, l] = x[c2*C2 + l, kd*128 + p]
    # q/k feed only the feature-map matmuls -> fp8 (error-tolerant path);
    # v feeds values -> f16.
    q_d = nc.dram_tensor("qb3", [NC2, 128, KDc, C2], f8, kind="ExternalInput").ap()
    k_d = nc.dram_tensor("kb3", [NC2, 128, KDc, C2], f8, kind="ExternalInput").ap()
    v_d = nc.dram_tensor("vb3", [NC2, 128, KDc, C2], f16, kind="ExternalInput").ap()
    # padded: head h's F features at columns 32h..32h+16 (32-aligned bases)
    # w2 pre-scaled by 16 on the host to dodge fp8 subnormals
    w2q_d = nc.dram_tensor("w2q", [D, HC * 32], f8, kind="ExternalInput").ap()
    w2k_d = nc.dram_tensor("w2k", [D, HC * 32], f8, kind="ExternalInput").ap()
    wvT_d = nc.dram_tensor("wvT", [D, M], f16, kind="ExternalInput").ap()
    woT_d = nc.dram_tensor("woT", [M, D], f16, kind="ExternalInput").ap()
    out_d = nc.dram_tensor("out", [L, D], f16, kind="ExternalOutput").ap()

    KD = D // 128  # 8 contraction blocks

    with tile.TileContext(nc) as tc, ExitStack() as ctx:
        consts = ctx.enter_context(tc.tile_pool(name="consts", bufs=1))
        io = ctx.enter_context(tc.tile_pool(name="io", bufs=3))
        io_v = ctx.enter_context(tc.tile_pool(name="io_v", bufs=2))
        work = ctx.enter_context(tc.tile_pool(name="work", bufs=2))
        small = ctx.enter_context(tc.tile_pool(name="small", bufs=3))
        outp = ctx.enter_context(tc.tile_pool(name="outp", bufs=2))
        ps_s = ctx.enter_context(tc.tile_pool(name="ps_s", bufs=2, space="PSUM"))
        ps_w = ctx.enter_context(tc.tile_pool(name="ps_w", bufs=4, space="PSUM"))
        ps_o = ctx.enter_context(tc.tile_pool(name="ps_o", bufs=2, space="PSUM"))

        # --- constants ---
        w2q_sb = consts.tile([128, KD, HC * 32], f8)
        nc.sync.dma_start(out=w2q_sb, in_=w2q_d.rearrange("(o p) f -> p o f", p=128))
        w2k_sb = consts.tile([128, KD, HC * 32], f8)
        nc.sync.dma_start(out=w2k_sb, in_=w2k_d.rearrange("(o p) f -> p o f", p=128))
        wvT_sb = consts.tile([128, KD, M], f16)
        nc.sync.dma_start(out=wvT_sb, in_=wvT_d.rearrange("(o p) m -> p o m", p=128))
        woT_sb = consts.tile([128, M // 128, D], f16)
        nc.sync.dma_start(out=woT_sb, in_=woT_d.rearrange("(o p) n -> p o n", p=128))

        ident = consts.tile([128, 128], f16)
        make_identity(nc, ident)
        # causal mask over (j=key partition, i=query free): 1 where j <= i
        mask = consts.tile([128, 128], f32)
        nc.vector.memset(mask, 1.0)
        nc.gpsimd.affine_select(
            out=mask, in_=mask, pattern=[[1, 128]],
            compare_op=mybir.AluOpType.is_ge, fill=0.0,
            base=0, channel_multiplier=-1,
        )

        # persistent running state: [f, h, (v|1)] accumulated over chunks.
        # Kept in SBUF (not PSUM): multiple interleaved matmul accumulation
        # groups in one PSUM bank corrupt each other (start=True clears the
        # whole bank's has_written bits).
        state_acc = consts.tile([F, HC, DK + 1], f32)
        nc.vector.memset(state_acc, 0.0)

        # PE warm-up: dense dummy matmuls while the first XBAR loads stream in,
        # so the HAM clock gate reaches 8/8 before real compute begins
        warm_sb = consts.tile([128, 512], f16)
        nc.vector.memset(warm_sb, 0.0)
        for wi in range(16):
            warm_ps = ps_o.tile([128, 512], f32, tag="op")
            nc.tensor.matmul(
                warm_ps, lhsT=warm_sb[:, 0:128], rhs=warm_sb,
                start=True, stop=True,
            )

        for c2 in range(NC2):
            l0 = c2 * C2
            qT = io.tile([128, KD, C2], f8, tag="qT")
            kT = io.tile([128, KD, C2], f8, tag="kT")
            vT = io_v.tile([128, KD, C2], f16, tag="vT")
            # contiguous pre-tiled loads, split across queues
            for kk in range(0, KD, 4):
                nc.sync.dma_start(out=qT[:, kk:kk + 4, :], in_=q_d[c2, :, kk:kk + 4, :])
                nc.sync.dma_start(out=kT[:, kk:kk + 4, :], in_=k_d[c2, :, kk:kk + 4, :])
            for kk in range(0, KD, 2):
                nc.sync.dma_start(out=vT[:, kk:kk + 2, :], in_=v_d[c2, :, kk:kk + 2, :])

            # --- fused feature projection: s16 = 16*s = W2_16^T @ xT, fp8
            # DoubleRow (2 k-tiles per matmul, 0.5 cycles/row)
            sq_ps = ps_s.tile([128, C2], f32, tag="s")
            sk_ps = ps_s.tile([128, C2], f32, tag="s")
            for j in range(KD // 2):
                nc.tensor.matmul(
                    sq_ps, lhsT=w2q_sb[:, 2 * j:2 * j + 2, :],
                    rhs=qT[:, 2 * j:2 * j + 2, :],
                    start=(j == 0), stop=(j == KD // 2 - 1),
                    perf_mode=mybir.MatmulPerfMode.DoubleRow,
                )
            for j in range(KD // 2):
                nc.tensor.matmul(
                    sk_ps, lhsT=w2k_sb[:, 2 * j:2 * j + 2, :],
                    rhs=kT[:, 2 * j:2 * j + 2, :],
                    start=(j == 0), stop=(j == KD // 2 - 1),
                    perf_mode=mybir.MatmulPerfMode.DoubleRow,
                )

            sq_sq = work.tile([128, C2], f32, tag="sq_sq")
            nc.scalar.square(sq_sq, sq_ps)
            sk_sq = work.tile([128, C2], f32, tag="sk_sq")
            nc.scalar.square(sk_sq, sk_ps)

            # per-head feature maps exp(-0.5 s^2), transposed [f, l], each in
            # its own base-0 tile (PE requires K=16 operands at partition 0)
            qpT = []
            kpT = []
            for h in range(HC):
                qp_h = small.tile([F, C2], f16, tag=f"qpT{h}")
                nc.scalar.activation(
                    qp_h, sq_sq[h * 32:h * 32 + F, :],
                    mybir.ActivationFunctionType.Exp, scale=-0.5 / 256.0,
                )
                qpT.append(qp_h)
                kp_h = small.tile([F, C2], f16, tag=f"kpT{h}")
                nc.scalar.activation(
                    kp_h, sk_sq[h * 32:h * 32 + F, :],
                    mybir.ActivationFunctionType.Exp, scale=-0.5 / 256.0,
                )
                kpT.append(kp_h)

            # --- scan, phase-major across the 4 subs: PE gets dense bursts
            # while DVE/ACT consumers of phase N overlap PE's phase N+1 ---

            # P1: kp natural [j, f] via PE transposes (identity [I16|0] zeroes pads)
            knat_pss = []
            for sub in range(SUB):
                ls = sub * C
                knat_ps = ps_w.tile([128, HC * 32], f16, tag="w")
                for h in range(HC):
                    nc.tensor.transpose(
                        knat_ps[:, h * 32:(h + 1) * 32],
                        kpT[h][:, ls:ls + C],
                        ident[:F, :32],
                    )
                knat_pss.append(knat_ps)

            # P2: knat to SBUF; b = 1/(sum_f kp + EPS)
            knats, b4s = [], []
            for sub in range(SUB):
                knat = small.tile([128, HC * 32], f16, tag="knat", bufs=5)
                nc.scalar.copy(knat, knat_pss[sub])
                knats.append(knat)
                bsum = small.tile([128, HC], f32, tag="bsum", bufs=5)
                nc.vector.reduce_sum(
                    out=bsum,
                    in_=knat_pss[sub].rearrange("p (h f) -> p h f", h=HC),
                    axis=mybir.AxisListType.X,
                )
                b4 = small.tile([128, HC], f32, tag="b4", bufs=5)
                nc.vector.tensor_scalar_add(b4, bsum, EPS)
                nc.vector.reciprocal(b4, b4)
                b4s.append(b4)

            # P3+P4: vh natural [l, m]; v1 = b * [v | 1]
            v1s = []
            for sub in range(SUB):
                ls = sub * C
                vh_ps = ps_w.tile([128, M], f32, tag="w")
                for kd in range(KD):
                    nc.tensor.matmul(
                        vh_ps, lhsT=vT[:, kd, ls:ls + C], rhs=wvT_sb[:, kd, :],
                        start=(kd == 0), stop=(kd == KD - 1),
                    )
                v1 = small.tile([128, HC, DK + 1], f16, tag="v1", bufs=6)
                nc.vector.tensor_tensor(
                    v1[:, :, 0:DK],
                    vh_ps.rearrange("p (h d) -> p h d", h=HC),
                    b4s[sub][:, :, None].to_broadcast((128, HC, DK)),
                    mybir.AluOpType.mult,
                )
                nc.vector.tensor_copy(v1[:, :, DK], b4s[sub])
                v1s.append(v1)

            # P5: state deltas: ONE matmul per sub for all heads; the
            # off-diagonal (h, h') blocks are computed but never read
            d_pss = []
            for sub in range(SUB):
                c = c2 * SUB + sub
                if c < NCH - 1:
                    d_ps = ps_w.tile([HC * 32, HC * (DK + 1)], f32, tag="w")
                    nc.tensor.matmul(
                        d_ps, lhsT=knats[sub],
                        rhs=v1s[sub].rearrange("p h n -> p (h n)"),
                        start=True, stop=True,
                    )
                    d_pss.append(d_ps)
                else:
                    d_pss.append(None)

            # P6: state prefix chain (f16 snapshot before each chunk)
            st16s = []
            for sub in range(SUB):
                c = c2 * SUB + sub
                st16 = small.tile([F, HC, DK + 1], f16, tag="st16", bufs=6)
                if c > 0:
                    nc.vector.tensor_copy(st16, state_acc)
                st16s.append(st16)
                if c < NCH - 1:
                    for h in range(HC):
                        nc.vector.tensor_add(
                            state_acc[:, h, :], state_acc[:, h, :],
                            d_pss[sub][h * 32:h * 32 + F,
                                       h * (DK + 1):(h + 1) * (DK + 1)],
                        )

            # P7+P8: A for all heads into one bank/sub; one mask op/sub
            a_ms = []
            for sub in range(SUB):
                ls = sub * C
                a_all = ps_w.tile([128, HC, C], f32, tag="w")
                for h in range(HC):
                    nc.tensor.matmul(
                        a_all[:, h, :],
                        lhsT=kpT[h][:, ls:ls + C], rhs=qpT[h][:, ls:ls + C],
                        start=True, stop=True,
                    )
                a_m = small.tile([128, HC, C], f16, tag="a_m", bufs=5)
                nc.vector.tensor_tensor(
                    a_m, a_all,
                    mask[:, None, :].to_broadcast((128, HC, C)),
                    mybir.AluOpType.mult,
                )
                a_ms.append(a_m)

            # P9+P10: o = A_m^T @ v1 (+ qp^T @ state); batched epilogue
            ohs = []
            for sub in range(SUB):
                c = c2 * SUB + sub
                ls = sub * C
                o_all = ps_w.tile([128, HC, DK + 1], f32, tag="w")
                for h in range(HC):
                    nc.tensor.matmul(
                        o_all[:, h, :], lhsT=a_ms[sub][:, h, :],
                        rhs=v1s[sub][:, h, :],
                        start=True, stop=(c == 0),
                    )
                    if c > 0:
                        nc.tensor.matmul(
                            o_all[:, h, :],
                            lhsT=qpT[h][:, ls:ls + C],
                            rhs=st16s[sub][:, h, :],
                            start=False, stop=True,
                        )
                r4 = small.tile([128, HC], f32, tag="r4", bufs=5)
                nc.vector.tensor_scalar_add(r4, o_all[:, :, DK], EPS)
                nc.vector.reciprocal(r4, r4)
                oh = outp.tile([128, M], f16, tag="oh", bufs=5)
                nc.vector.tensor_tensor(
                    oh.rearrange("p (h d) -> p h d", h=HC),
                    o_all[:, :, 0:DK],
                    r4[:, :, None].to_broadcast((128, HC, DK)),
                    mybir.AluOpType.mult,
                )
                ohs.append(oh)

            # P11: transpose oh -> [m, l]
            ohTs = []
            for sub in range(SUB):
                ohT = outp.tile([128, M // 128, C], f16, tag="ohT", bufs=5)
                for mb in range(M // 128):
                    ohT_ps = ps_w.tile([128, C], f16, tag="w")
                    nc.tensor.transpose(
                        ohT_ps, ohs[sub][:, mb * 128:(mb + 1) * 128], ident,
                    )
                    nc.scalar.copy(ohT[:, mb, :], ohT_ps)
                ohTs.append(ohT)

            # P12+P13: output projection, copy out, DMA
            for sub in range(SUB):
                ls = sub * C
                out_sb = outp.tile([128, D], f16, tag="out_sb", bufs=3)
                for nh in range(2):
                    op_ps = ps_o.tile([128, D // 2], f32, tag="op")
                    for mb in range(M // 128):
                        nc.tensor.matmul(
                            op_ps, lhsT=ohTs[sub][:, mb, :],
                            rhs=woT_sb[:, mb, nh * 512:(nh + 1) * 512],
                            start=(mb == 0), stop=(mb == M // 128 - 1),
                        )
                    if nh == 0:
                        nc.vector.tensor_copy(out_sb[:, 0:512], op_ps)
                    else:
                        nc.scalar.copy(out_sb[:, 512:1024], op_ps)
                nc.scalar.dma_start(out=out_d[l0 + ls:l0 + ls + C, :], in_=out_sb)

    nc.compile()
    return nc


def _get_nc():
    if "nc" not in _CACHE:
        _CACHE["nc"] = _build_bass()
    return _CACHE["nc"]


def _f8(x):
    import ml_dtypes
    return np.ascontiguousarray(
        np.clip(np.asarray(x, np.float32), -240, 240).astype(ml_dtypes.float8_e4m3)
    )


def make_in_maps(q, k, v, w_q, w_k, w_v, w_o, omega):
    B = q.shape[0]
    qk8 = {}
    for b in range(B):
        qk8[b] = (_tile_T(_f8(q[b])), _tile_T(_f8(k[b])))
    in_maps = []
    for core in range(N_CORES):
        b = core // (N_CORES // B)
        g = core % (N_CORES // B)
        rows = slice(g * M, (g + 1) * M)
        om = omega.astype(np.float64)
        w2q = np.zeros((D, HC * 32), np.float64)
        w2k = np.zeros((D, HC * 32), np.float64)
        for h in range(HC):
            wq_h = w_q[rows][h * DK:(h + 1) * DK].astype(np.float64)  # [DK, D]
            wk_h = w_k[rows][h * DK:(h + 1) * DK].astype(np.float64)
            w2q[:, h * 32:h * 32 + F] = (om @ wq_h).T  # [D, F]
            w2k[:, h * 32:h * 32 + F] = (om @ wk_h).T
        in_maps.append({
            "qb3": qk8[b][0],
            "kb3": qk8[b][1],
            "vb3": _tile_T(np.asarray(v[b], np.float16)),
            "w2q": _f8(w2q * 16.0),
            "w2k": _f8(w2k * 16.0),
            "wvT": np.ascontiguousarray(w_v[rows].T).astype(np.float16),
            "woT": np.ascontiguousarray(w_o[:, rows].T).astype(np.float16),
        })
    return in_maps


def kernel(q, k, v, w_q, w_k, w_v, w_o, omega):
    from concourse.bass_utils import run_bass_kernel_spmd

    B = q.shape[0]
    nc = _get_nc()
    in_maps = make_in_maps(q, k, v, w_q, w_k, w_v, w_o, omega)
    res = run_bass_kernel_spmd(nc, in_maps, core_ids=list(range(N_CORES)))
    out = np.zeros((B, L, D), np.float32)
    for core in range(N_CORES):
        out[core // (N_CORES // B)] += res.results[core]["out"]
    return out

